# revision 2
# baseline (speedup 1.0000x reference)
"""Trainium2 Bass kernel for nn_Attention_3607772529228 (sparse_attention).

Reference computation (B=64, S=512, T=32, 2H=1024, ATT=512):
    ht_mean = mean(ht, axis=1)                               [B, 2H]
    z       = [h ; ht_mean] @ w1_w.T + w1_b                  [B, S, ATT]
    a       = tanh(z)
    beta    = a @ u_w[0];  beta = where(mask, beta, -1e20)   [B, S]
    alpha   = softmax(beta, axis=1)
    out     = einsum('bs,bsd->bd', alpha, h)                 [B, 2H]

Algebraic simplifications (exact):
  * where(valid,...) maskings of h_cat and `a` don't affect the output.
  * The ht_mean half of the big matmul folds into a per-batch bias:
    z = h @ w1.T + (w2 @ ht_mean + w1_b).
  * Invalid positions get alpha == 0 exactly, so each batch's sequence is
    host-COMPACTED to its valid positions (a permutation).

Precision: the z matmul runs in fp8 e4m3 (TRN FP8_EXP4) with
perf_mode=DoubleRow (2 k-rows per PE cell -> ~2x matmul throughput).
w2/ht (bias path) are fp8 as well.  The weighted-sum path keeps h in
bf16.  Simulated end-to-end rel err ~1e-2 vs the 2e-2 gate.

Distribution + raggedness: data-parallel, 8 batches/core.  Batches are
sorted by valid-count and dealt slot-wise across cores (slot j on core c
holds the (8j+c)-th largest batch), so per-SLOT widths
(SQ_j = ceil16(slot max count), VB_j, chunk counts C_j) are identical on
every core and can be baked into one SPMD NEFF.  The z/beta path runs on
[ATT, SQ_j] tiles; the weighted-sum path loads h naturally with exact
VB_j rows (last chunk partial-K matmul).
"""

import os
from contextlib import ExitStack

import numpy as np
import ml_dtypes

import concourse.bass as bass
import concourse.tile as tile
from concourse import bacc, mybir
from concourse import bass_utils
from concourse.masks import make_identity

BF16 = mybir.dt.bfloat16
F32 = mybir.dt.float32
F8 = mybir.dt.float8e4
NP_F8 = ml_dtypes.float8_e4m3  # TRN FP8_EXP4 (max +-240), NOT e4m3fn

B, S, T, H2, ATT = 64, 512, 32, 1024, 512
NCORES = 8
BL = B // NCORES  # 8 batch slots per core
P = 128
KC = H2 // P  # 8 k-chunks over hidden
K2 = KC // 2  # 4 DoubleRow k-pair chunks
TT = ATT // P  # 4 attention tiles
NH = H2 // 512  # 2 output halves
NG = BL // 4  # slot groups of 4 (PE column-group packing)
DR = mybir.MatmulPerfMode.DoubleRow


def _ceil(x, m):
    return -(-int(x) // m) * m


def plan_for_mask(h_mask):
    """Slot-wise widths + batch assignment. Identical widths across cores."""
    counts = (np.asarray(h_mask) != 0).sum(axis=1)
    order = np.argsort(-counts, kind="stable")
    vb = [int(counts[order[8 * j]]) for j in range(BL)]  # slot max count
    sq = [_ceil(v, 16) for v in vb]
    assert all(v <= 3 * P for v in vb) and max(sq) <= S
    return {
        "order": order,
        "counts": counts,
        "VB": tuple(vb),
        "SQ": tuple(sq),
    }


def _body(tc, reps, SQ, VB):
    SQmax = max(SQ)
    SNmax = _ceil(SQmax, P)
    SC = SNmax // P
    C = [-(-v // P) for v in VB]  # h_nat chunks per slot
    CO = np.concatenate([[0], np.cumsum(C)]).astype(int)  # chunk offsets
    QO = np.concatenate([[0], np.cumsum(SQ)]).astype(int)  # h_t col offsets
    RO = np.concatenate([[0], np.cumsum(VB)]).astype(int)  # h_nat row offsets
    SUMSQ, CT, SUMVB = int(QO[-1]), int(CO[-1]), int(RO[-1])
    nc = tc.nc
    ctx = tc._ctx

    ht_ap = nc.dram_tensor("h_t", [H2, SUMSQ], F8, kind="ExternalInput").ap()
    hn_ap = nc.dram_tensor("h_nat", [SUMVB, H2], BF16, kind="ExternalInput").ap()
    w1t_ap = nc.dram_tensor("w1t", [H2, ATT], F8, kind="ExternalInput").ap()
    w2t_ap = nc.dram_tensor("w2t", [H2, ATT], F8, kind="ExternalInput").ap()
    htt_ap = nc.dram_tensor("htt", [H2, BL * T], F8, kind="ExternalInput").ap()
    u_ap = nc.dram_tensor("u_col", [P, TT, 32], BF16, kind="ExternalInput").ap()
    w1b_ap = nc.dram_tensor("w1b_col", [P, TT], F32, kind="ExternalInput").ap()
    mask_ap = nc.dram_tensor("maskadd", [BL, SQmax], F32, kind="ExternalInput").ap()
    out_ap = nc.dram_tensor("out", [BL, H2], F32, kind="ExternalOutput").ap()

    singles = ctx.enter_context(tc.tile_pool(name="singles", bufs=1))
    hT_pool = ctx.enter_context(tc.tile_pool(name="hT", bufs=8))
    a_pool = ctx.enter_context(tc.tile_pool(name="a", bufs=20))
    rows = ctx.enter_context(tc.tile_pool(name="rows", bufs=4))
    z_psum = ctx.enter_context(tc.tile_pool(name="z_ps", bufs=1, space="PSUM"))
    beta_psum = ctx.enter_context(tc.tile_pool(name="beta_ps", bufs=1, space="PSUM"))
    aT_psum = ctx.enter_context(tc.tile_pool(name="aT_ps", bufs=1, space="PSUM"))
    ws_psum = ctx.enter_context(tc.tile_pool(name="ws_ps", bufs=2, space="PSUM"))

    def emit():
        hT_tiles = [None] * BL
        h_nat = singles.tile([P, CT, H2], BF16)

        def load_slot(j):
            # z-path copy: [P, KC, SQ_j] fp8, k-chunk-major over partitions
            hT_j = hT_pool.tile([P, KC, SQ[j]], F8, tag="hT")
            nc.sync.dma_start(
                out=hT_j,
                in_=ht_ap[:, QO[j] : QO[j] + SQ[j]].rearrange(
                    "(c p) s -> p c s", p=P
                ),
            )
            hT_tiles[j] = hT_j
            # ws-path copy: natural layout, exact VB_j rows (ragged last chunk)
            for c in range(C[j]):
                k = min(P, VB[j] - P * c)
                nc.sync.dma_start(
                    out=h_nat[0:k, CO[j] + c, :],
                    in_=hn_ap[RO[j] + P * c : RO[j] + P * c + k, :],
                )

        for j in range(4):
            load_slot(j)
        w1t_sb = singles.tile([P, KC, ATT], F8)
        nc.sync.dma_start(out=w1t_sb, in_=w1t_ap.rearrange("(c p) a -> p c a", p=P))

        u_sb = singles.tile([P, TT, 32], BF16)
        nc.sync.dma_start(out=u_sb, in_=u_ap)
        w1b_sb = singles.tile([P, TT], F32)
        nc.sync.dma_start(out=w1b_sb, in_=w1b_ap)
        mask_sb = singles.tile([BL, SQmax], F32)
        nc.sync.dma_start(out=mask_sb, in_=mask_ap)
        ident = singles.tile([P, P], BF16)
        make_identity(nc, ident)
        w2t_sb = singles.tile([P, KC, ATT], F8)
        nc.sync.dma_start(out=w2t_sb, in_=w2t_ap.rearrange("(c p) a -> p c a", p=P))

        # ---- ht mean -> per-slot bias columns ----
        htm = singles.tile([P, KC, BL], F8)
        htT_sb = singles.tile([P, KC, BL * T], F8)
        nc.sync.dma_start(out=htT_sb, in_=htt_ap.rearrange("(c p) j -> p c j", p=P))
        for c in range(KC):
            with nc.allow_low_precision("fp8 sum of 32 fp8 values, fp32 internal"):
                nc.vector.reduce_sum(
                    out=htm[:, c, :],
                    in_=htT_sb[:, c, :].rearrange("p (b t) -> p b t", b=BL),
                    axis=mybir.AxisListType.X,
                )

        # bias_col[t] = (w2 @ ht_sum)/T + w1_b   ([128, BL] per att tile)
        bias_col = singles.tile([P, TT, BL], F32)
        for t in range(TT):
            b2_ps = ws_psum.tile([P, 512], F32, tag="ws")
            for c in range(KC):
                nc.tensor.matmul(
                    b2_ps[:, 0:BL],
                    lhsT=w2t_sb[:, c, t * P : (t + 1) * P],
                    rhs=htm[:, c, :],
                    start=(c == 0),
                    stop=(c == KC - 1),
                )
            nc.vector.tensor_scalar(
                out=bias_col[:, t, :],
                in0=b2_ps[:, 0:BL],
                scalar1=1.0 / T,
                scalar2=w1b_sb[:, t : t + 1],
                op0=mybir.AluOpType.mult,
                op1=mybir.AluOpType.add,
            )

        beta_all = singles.tile([BL, SQmax], F32)
        nc.vector.memset(beta_all, 0.0)

        # ---- main pipeline: z (fp8 DoubleRow) + tanh; w1 pair-chunks stay
        # stationary while the 4 slots of a group stream through ----
        a_tiles = {}
        for g in range(NG):
            if g + 1 < NG:
                for j in range(4 * (g + 1), 4 * (g + 2)):
                    load_slot(j)
            for t in range(TT):
                z_tiles = [
                    z_psum.tile([P, SQmax], F32, tag=f"z{bb}", bufs=1, name=f"z_{bb}")
                    for bb in range(4)
                ]
                for k2 in range(K2):
                    for bb in range(4):
                        j = 4 * g + bb
                        nc.tensor.matmul(
                            z_tiles[bb][:, 0 : SQ[j]],
                            lhsT=w1t_sb.rearrange("p (k i) a -> p k i a", i=2)[
                                :, k2, :, t * P : (t + 1) * P
                            ],
                            rhs=hT_tiles[j].rearrange("p (k i) s -> p k i s", i=2)[
                                :, k2, :, :
                            ],
                            start=(k2 == 0),
                            stop=(k2 == K2 - 1),
                            perf_mode=DR,
                        )
                for bb in range(4):
                    j = 4 * g + bb
                    a_t = a_pool.tile([P, SQmax], BF16, tag="a")
                    nc.scalar.activation(
                        out=a_t[:, 0 : SQ[j]],
                        in_=z_tiles[bb][:, 0 : SQ[j]],
                        func=mybir.ActivationFunctionType.Tanh,
                        bias=bias_col[:, t, j : j + 1],
                        scale=1.0,
                    )
                    a_tiles[(j, t)] = a_t
            # beta for the 4 slots of this group, one PE column group each
            beta_ps = beta_psum.tile([P, SQmax], F32, tag="beta")
            for bb in range(4):
                j = 4 * g + bb
                for t in range(TT):
                    nc.tensor.matmul(
                        beta_ps[32 * bb : 32 * bb + 32, 0 : SQ[j]],
                        lhsT=u_sb[:, t, :],
                        rhs=a_tiles[(j, t)][:, 0 : SQ[j]],
                        start=(t == 0),
                        stop=(t == TT - 1),
                        tile_position=(0, 32 * bb),
                    )
            beta_sc = rows.tile([P, SQmax], F32, tag="betarow")
            for bb in range(4):
                j = 4 * g + bb
                nc.vector.tensor_copy(
                    out=beta_sc[32 * bb : 32 * bb + 32, 0 : SQ[j]],
                    in_=beta_ps[32 * bb : 32 * bb + 32, 0 : SQ[j]],
                )
                nc.gpsimd.dma_start(
                    out=beta_all[j : j + 1, 0 : SQ[j]],
                    in_=beta_sc[32 * bb : 32 * bb + 1, 0 : SQ[j]],
                )

        # ---- softmax over the free dim for all 8 slots at once ----
        beta_m = singles.tile([BL, SQmax], F32)
        nc.vector.tensor_add(beta_m, beta_all, mask_sb)
        negmax = singles.tile([BL, 1], F32)
        nc.vector.reduce_max(
            out=negmax, in_=beta_m, axis=mybir.AxisListType.X, negate=True
        )
        sumrow = singles.tile([BL, 1], F32)
        alpha_bf = singles.tile([BL, SNmax], BF16)
        nc.vector.memset(alpha_bf[:, SQmax:SNmax], 0.0)
        ex = singles.tile([BL, SQmax], F32)
        nc.scalar.activation(
            out=ex,
            in_=beta_m,
            func=mybir.ActivationFunctionType.Exp,
            bias=negmax[:, 0:1],
            scale=1.0,
            accum_out=sumrow[:, 0:1],
        )
        rinv = singles.tile([BL, 1], F32)
        nc.vector.reciprocal(rinv, sumrow)
        nc.vector.tensor_scalar_mul(alpha_bf[:, 0:SQmax], ex, rinv[:, 0:1])

        # ---- transpose alpha: [BL, SNmax] -> SC x [128, BL] via PE ----
        alpha_rep = singles.tile([P, SC, BL, 32], BF16)
        for sc in range(SC):
            aT_ps = aT_psum.tile([P, BL], BF16, tag="aT")
            nc.tensor.transpose(
                aT_ps,
                alpha_bf[0:BL, sc * P : (sc + 1) * P],
                ident[0:BL, 0:BL],
            )
            aT_bcast = bass.AP(
                tensor=aT_ps.tensor,
                offset=aT_ps.offset,
                ap=[aT_ps.ap[0], aT_ps.ap[1], [0, 32]],
            )
            nc.vector.tensor_copy(out=alpha_rep[:, sc, :, :], in_=aT_bcast)

        # ---- weighted sum, 4 slots packed in PE column groups, ragged K ----
        for g in range(NG):
            for nh in range(NH):
                ws_ps = ws_psum.tile([P, 512], F32, tag="ws")
                for bb in range(4):
                    j = 4 * g + bb
                    for c in range(C[j]):
                        k = min(P, VB[j] - P * c)
                        nc.tensor.matmul(
                            ws_ps[32 * bb : 32 * bb + 32, :],
                            lhsT=alpha_rep[0:k, c, j, :],
                            rhs=h_nat[0:k, CO[j] + c, nh * 512 : (nh + 1) * 512],
                            start=(c == 0),
                            stop=(c == C[j] - 1),
                            tile_position=(0, 32 * bb),
                        )
                o_sc = rows.tile([P, 512], F32, tag="orow")
                nc.vector.tensor_copy(out=o_sc, in_=ws_ps)
                nc.gpsimd.dma_start(
                    out=out_ap[4 * g : 4 * g + 4, nh * 512 : (nh + 1) * 512],
                    in_=o_sc.rearrange("(b r) s -> b r s", r=32)[:, 0, :],
                )

    for _rep in range(reps):
        emit()


_CACHE = {}


def build(reps=1, plan=None):
    key = ("nc", reps, plan["SQ"], plan["VB"])
    if key in _CACHE:
        return _CACHE[key]
    nc = bacc.Bacc("TRN2", target_bir_lowering=False, debug=False)
    with tile.TileContext(nc) as tc:
        with ExitStack() as ctx:
            tc._ctx = ctx
            _body(tc, reps, plan["SQ"], plan["VB"])
    nc.compile()
    _CACHE[key] = nc
    return nc


def _prep_core_inputs(h, h_mask, ht, w1_w, w1_b, u_w, plan=None):
    """Host-side sharding + layout prep. Returns list of 8 per-core dicts."""
    if plan is None:
        plan = plan_for_mask(h_mask)
    order, counts, VB, SQ = plan["order"], plan["counts"], plan["VB"], plan["SQ"]
    SQmax = max(SQ)
    SUMSQ, SUMVB = sum(SQ), sum(VB)
    QO = np.concatenate([[0], np.cumsum(SQ)]).astype(int)
    RO = np.concatenate([[0], np.cumsum(VB)]).astype(int)
    bf = ml_dtypes.bfloat16
    h_f = np.asarray(h, dtype=np.float32)
    ht_f = np.asarray(ht, dtype=np.float32)
    mask = np.asarray(h_mask) != 0

    w1t = np.ascontiguousarray(np.asarray(w1_w[:, :H2], np.float32).T).astype(NP_F8)
    w2t = np.ascontiguousarray(np.asarray(w1_w[:, H2:], np.float32).T).astype(NP_F8)
    u_col = np.ascontiguousarray(
        np.repeat(
            np.asarray(u_w[0], dtype=np.float32).reshape(TT, P).T[:, :, None],
            32,
            axis=2,
        )
    ).astype(bf)
    w1b_col = np.ascontiguousarray(
        np.asarray(w1_b, dtype=np.float32).reshape(TT, P).T
    ).astype(np.float32)

    in_maps = []
    for core in range(NCORES):
        h_t = np.zeros((H2, SUMSQ), dtype=NP_F8)
        h_nat = np.zeros((SUMVB, H2), dtype=bf)
        maskadd = np.full((BL, SQmax), -1.0e20, dtype=np.float32)
        ht_sel = np.empty((BL, T, H2), dtype=np.float32)
        for j in range(BL):
            b = order[8 * j + core]
            idx = np.flatnonzero(mask[b])
            v = len(idx)
            assert v <= VB[j], f"core {core} slot {j}: {v} > VB {VB[j]}"
            hc = h_f[b, idx]  # [v, H2]
            h_t[:, QO[j] : QO[j] + v] = hc.astype(NP_F8).T
            h_nat[RO[j] : RO[j] + v] = hc.astype(bf)
            maskadd[j, :v] = 0.0
            ht_sel[j] = ht_f[b]
        htt = np.ascontiguousarray(ht_sel.reshape(BL * T, H2).T).astype(NP_F8)
        in_maps.append(
            {
                "h_t": np.ascontiguousarray(h_t),
                "h_nat": np.ascontiguousarray(h_nat),
                "w1t": w1t,
                "w2t": w2t,
                "htt": htt,
                "u_col": u_col,
                "w1b_col": w1b_col,
                "maskadd": maskadd,
            }
        )
    return in_maps


def kernel(h, h_mask, ht, w1_w, w1_b, u_w):
    plan = plan_for_mask(h_mask)
    nc = build(plan=plan)
    in_maps = _prep_core_inputs(h, h_mask, ht, w1_w, w1_b, u_w, plan=plan)
    res = bass_utils.run_bass_kernel_spmd(
        nc,
        in_maps,
        core_ids=list(range(NCORES)),
        trace=bool(int(os.environ.get("KERNEL_TRACE", "0"))),
    )
    _CACHE["last_result"] = res
    out = np.empty((B, H2), dtype=np.float32)
    order = plan["order"]
    for core in range(NCORES):
        for j in range(BL):
            out[order[8 * j + core]] = res.results[core]["out"][j]
    return np.ascontiguousarray(out)


# revision 8
# speedup vs baseline: 2.2906x; 2.2906x over previous
"""Trainium2 Bass kernel for nn_Attention_3607772529228 (sparse_attention).

Reference computation (B=64, S=512, T=32, 2H=1024, ATT=512):
    ht_mean = mean(ht, axis=1)                               [B, 2H]
    z       = [h ; ht_mean] @ w1_w.T + w1_b                  [B, S, ATT]
    a       = tanh(z)
    beta    = a @ u_w[0];  beta = where(mask, beta, -1e20)   [B, S]
    alpha   = softmax(beta, axis=1)
    out     = einsum('bs,bsd->bd', alpha, h)                 [B, 2H]

Algebraic simplifications (exact):
  * where(valid,...) maskings of h_cat and `a` don't affect the output.
  * The ht_mean half of the big matmul folds into a per-batch bias:
    z = h @ w1.T + (w2 @ ht_mean + w1_b).
  * Invalid positions get alpha == 0 exactly, so each batch's sequence is
    host-COMPACTED to its valid positions (a permutation).

Precision: the z matmul runs in fp8 e4m3 (TRN FP8_EXP4) with
perf_mode=DoubleRow (2 k-rows per PE cell -> ~2x matmul throughput).
w2/ht (bias path) are fp8 as well.  The weighted-sum path keeps h in
bf16.  End-to-end rel err ~1e-2 vs the 2e-2 gate.

Distribution + raggedness: data-parallel, 8 batch slots/core.  Batches
are sorted by valid-count and dealt slot-wise across cores, so per-GROUP
widths (WG_g = ceil16 of the group max count) and per-slot row counts
VB_j are identical on every core -> one SPMD NEFF with the ragged sizes
baked in.  DMA layouts are partition-major with per-partition runs well
above 512B, and transfers are batched into few large DMAs (hT: one per
4-slot group; w1/w2/ht: one fused; h_nat full chunks: one).
"""

import os
from contextlib import ExitStack

import numpy as np
import ml_dtypes

import concourse.bass as bass
import concourse.tile as tile
from concourse import bacc, mybir
from concourse import bass_utils
from concourse.masks import make_identity

BF16 = mybir.dt.bfloat16
F32 = mybir.dt.float32
F8 = mybir.dt.float8e4
NP_F8 = ml_dtypes.float8_e4m3  # TRN FP8_EXP4 (max +-240), NOT e4m3fn

B, S, T, H2, ATT = 64, 512, 32, 1024, 512
NCORES = 8
BL = B // NCORES  # 8 batch slots per core
P = 128
KC = H2 // P  # 8 k-chunks over hidden
K2 = KC // 2  # 4 DoubleRow k-pair chunks
TT = ATT // P  # 4 attention tiles
NH = H2 // 512  # 2 output halves
NG = BL // 4  # slot groups of 4 (PE column-group packing)
WF = 2 * ATT + BL * T  # fused weights free width (w1 | w2 | htT)
DR = mybir.MatmulPerfMode.DoubleRow


def _ceil(x, m):
    return -(-int(x) // m) * m


def plan_for_mask(h_mask):
    """Group widths + slot row counts + batch assignment (same on all cores)."""
    counts = (np.asarray(h_mask) != 0).sum(axis=1)
    order = np.argsort(-counts, kind="stable")
    vb = [int(counts[order[8 * j]]) for j in range(BL)]  # slot max count
    wg = [_ceil(max(vb[4 * g : 4 * g + 4]), 16) for g in range(NG)]
    assert all(v <= 3 * P for v in vb) and max(wg) <= S
    return {"order": order, "counts": counts, "VB": tuple(vb), "WG": tuple(wg)}


def _body(tc, reps, WG, VB):
    SQmax = max(WG)
    SNmax = _ceil(SQmax, P)
    SC = SNmax // P
    FC = [v // P for v in VB]  # full h_nat chunks per slot
    REM = [VB[j] - P * FC[j] for j in range(BL)]  # partial chunk rows
    FO = np.concatenate([[0], np.cumsum(FC)]).astype(int)
    FCT = int(FO[-1])  # total full chunks
    CT = FCT + BL  # + one (possibly empty) partial chunk slot each
    PO = np.concatenate([[0], np.cumsum(REM)]).astype(int)  # h_natp row offs
    XO = [0, KC * 4 * WG[0]]  # h_t col offsets per group
    SUMX = KC * 4 * (WG[0] + WG[1])
    nc = tc.nc
    ctx = tc._ctx

    ht_ap = nc.dram_tensor("h_t", [P, SUMX], F8, kind="ExternalInput").ap()
    hnf_ap = nc.dram_tensor("h_natf", [P, FCT * H2], BF16, kind="ExternalInput").ap()
    hnp_ap = nc.dram_tensor("h_natp", [int(PO[-1]), H2], BF16, kind="ExternalInput").ap()
    wts_ap = nc.dram_tensor("wts", [P, KC * WF], F8, kind="ExternalInput").ap()
    u_ap = nc.dram_tensor("u_col", [P, TT, 32], BF16, kind="ExternalInput").ap()
    w1b_ap = nc.dram_tensor("w1b_col", [P, TT], F32, kind="ExternalInput").ap()
    mask_ap = nc.dram_tensor("maskadd", [BL, SQmax], F32, kind="ExternalInput").ap()
    out_ap = nc.dram_tensor("out", [BL, H2], F32, kind="ExternalOutput").ap()

    # bufs=2 ping-pongs every per-rep tile across reps, so rep r+1's input
    # DMAs overlap rep r's tail compute instead of waiting on buffer reuse.
    singles = ctx.enter_context(tc.tile_pool(name="singles", bufs=2))
    hT_pool = ctx.enter_context(tc.tile_pool(name="hT", bufs=4))
    a_pool = ctx.enter_context(tc.tile_pool(name="a", bufs=20))
    rows = ctx.enter_context(tc.tile_pool(name="rows", bufs=4))
    z_psum = ctx.enter_context(tc.tile_pool(name="z_ps", bufs=1, space="PSUM"))
    beta_psum = ctx.enter_context(tc.tile_pool(name="beta_ps", bufs=1, space="PSUM"))
    aT_psum = ctx.enter_context(tc.tile_pool(name="aT_ps", bufs=1, space="PSUM"))
    ws_psum = ctx.enter_context(tc.tile_pool(name="ws_ps", bufs=2, space="PSUM"))

    def emit():
        # ---- loads: big batched partition-major DMAs ----
        # group 0 hT on SP; group 1 hT on Activation HWDGE queue (parallel
        # descriptor-gen, transfer ordering handled by deps)
        hT_g = [None] * NG

        def load_group(g, eng):
            t_ = hT_pool.tile([P, KC, 4 * WG[g]], F8, tag="hT")
            eng.dma_start(
                out=t_,
                in_=ht_ap[:, XO[g] : XO[g] + KC * 4 * WG[g]].rearrange(
                    "p (c x) -> p c x", c=KC
                ),
            )
            hT_g[g] = t_

        wts_sb = singles.tile([P, KC, WF], F8)
        nc.sync.dma_start(out=wts_sb, in_=wts_ap.rearrange("p (c f) -> p c f", c=KC))
        load_group(0, nc.sync)
        load_group(1, nc.scalar)

        u_sb = singles.tile([P, TT, 32], BF16)
        nc.sync.dma_start(out=u_sb, in_=u_ap)
        w1b_sb = singles.tile([P, TT], F32)
        nc.sync.dma_start(out=w1b_sb, in_=w1b_ap)
        mask_sb = singles.tile([BL, SQmax], F32)
        nc.sync.dma_start(out=mask_sb, in_=mask_ap)
        ident = singles.tile([P, P], BF16)
        make_identity(nc, ident)

        # h natural layout: all full chunks in one DMA, partials per slot
        h_nat = singles.tile([P, CT, H2], BF16)
        nc.sync.dma_start(
            out=h_nat[:, 0:FCT, :],
            in_=hnf_ap.rearrange("p (c d) -> p c d", d=H2),
        )
        for j in range(BL):
            if REM[j]:
                # SWDGE (Pool) path: keeps the tiny ragged-tail transfers off
                # the shared HWDGE descriptor generator
                nc.gpsimd.dma_start(
                    out=h_nat[0 : REM[j], FCT + j, :],
                    in_=hnp_ap[PO[j] : PO[j] + REM[j], :],
                )

        # ---- ht mean -> per-slot bias columns ----
        htm = singles.tile([P, KC, BL], F8)
        for c in range(KC):
            with nc.allow_low_precision("fp8 sum of 32 fp8 values, fp32 internal"):
                nc.vector.reduce_sum(
                    out=htm[:, c, :],
                    in_=wts_sb[:, c, 2 * ATT : WF].rearrange(
                        "p (b t) -> p b t", b=BL
                    ),
                    axis=mybir.AxisListType.X,
                )

        bias_col = singles.tile([P, TT, BL], F32)
        for t in range(TT):
            b2_ps = ws_psum.tile([P, 512], F32, tag="ws")
            for c in range(KC):
                nc.tensor.matmul(
                    b2_ps[:, 0:BL],
                    lhsT=wts_sb[:, c, ATT + t * P : ATT + (t + 1) * P],
                    rhs=htm[:, c, :],
                    start=(c == 0),
                    stop=(c == KC - 1),
                )
            nc.vector.tensor_scalar(
                out=bias_col[:, t, :],
                in0=b2_ps[:, 0:BL],
                scalar1=1.0 / T,
                scalar2=w1b_sb[:, t : t + 1],
                op0=mybir.AluOpType.mult,
                op1=mybir.AluOpType.add,
            )

        beta_all = singles.tile([BL, SQmax], F32)
        nc.vector.memset(beta_all, 0.0)

        # ---- main pipeline: z (fp8 DoubleRow) + tanh; w1 pair-chunks stay
        # stationary while the 4 slots of a group stream through ----
        a_tiles = {}
        for g in range(NG):
            W = WG[g]
            hT_r = hT_g[g].rearrange("p (k i) x -> p k i x", i=2)
            w1_r = wts_sb.rearrange("p (k i) f -> p k i f", i=2)
            for t in range(TT):
                z_tiles = [
                    z_psum.tile([P, SQmax], F32, tag=f"z{bb}", bufs=1, name=f"z_{bb}")
                    for bb in range(4)
                ]
                for k2 in range(K2):
                    for bb in range(4):
                        nc.tensor.matmul(
                            z_tiles[bb][:, 0:W],
                            lhsT=w1_r[:, k2, :, t * P : (t + 1) * P],
                            rhs=hT_r[:, k2, :, bb * W : (bb + 1) * W],
                            start=(k2 == 0),
                            stop=(k2 == K2 - 1),
                            perf_mode=DR,
                        )
                for bb in range(4):
                    j = 4 * g + bb
                    a_t = a_pool.tile([P, SQmax], BF16, tag="a")
                    nc.scalar.activation(
                        out=a_t[:, 0:W],
                        in_=z_tiles[bb][:, 0:W],
                        func=mybir.ActivationFunctionType.Tanh,
                        bias=bias_col[:, t, j : j + 1],
                        scale=1.0,
                    )
                    a_tiles[(j, t)] = a_t
            # beta for the 4 slots of this group, one PE column group each
            beta_ps = beta_psum.tile([P, SQmax], F32, tag="beta")
            for bb in range(4):
                j = 4 * g + bb
                for t in range(TT):
                    nc.tensor.matmul(
                        beta_ps[32 * bb : 32 * bb + 32, 0:W],
                        lhsT=u_sb[:, t, :],
                        rhs=a_tiles[(j, t)][:, 0:W],
                        start=(t == 0),
                        stop=(t == TT - 1),
                        tile_position=(0, 32 * bb),
                    )
            beta_sc = rows.tile([P, SQmax], F32, tag="betarow")
            nc.vector.tensor_copy(out=beta_sc[:, 0:W], in_=beta_ps[:, 0:W])
            nc.gpsimd.dma_start(
                out=beta_all[4 * g : 4 * g + 4, 0:W],
                in_=beta_sc.rearrange("(b r) s -> b r s", r=32)[:, 0, 0:W],
            )

        # ---- softmax over the free dim for all 8 slots at once ----
        beta_m = singles.tile([BL, SQmax], F32)
        nc.vector.tensor_add(beta_m, beta_all, mask_sb)
        negmax = singles.tile([BL, 1], F32)
        nc.vector.reduce_max(
            out=negmax, in_=beta_m, axis=mybir.AxisListType.X, negate=True
        )
        sumrow = singles.tile([BL, 1], F32)
        alpha_bf = singles.tile([BL, SNmax], BF16)
        nc.vector.memset(alpha_bf[:, SQmax:SNmax], 0.0)
        ex = singles.tile([BL, SQmax], F32)
        nc.scalar.activation(
            out=ex,
            in_=beta_m,
            func=mybir.ActivationFunctionType.Exp,
            bias=negmax[:, 0:1],
            scale=1.0,
            accum_out=sumrow[:, 0:1],
        )
        rinv = singles.tile([BL, 1], F32)
        nc.vector.reciprocal(rinv, sumrow)
        nc.vector.tensor_scalar_mul(alpha_bf[:, 0:SQmax], ex, rinv[:, 0:1])

        # ---- transpose alpha: [BL, SNmax] -> SC x [128, BL] via PE ----
        alpha_rep = singles.tile([P, SC, BL, 32], BF16)
        for sc in range(SC):
            aT_ps = aT_psum.tile([P, BL], BF16, tag="aT")
            nc.tensor.transpose(
                aT_ps,
                alpha_bf[0:BL, sc * P : (sc + 1) * P],
                ident[0:BL, 0:BL],
            )
            aT_bcast = bass.AP(
                tensor=aT_ps.tensor,
                offset=aT_ps.offset,
                ap=[aT_ps.ap[0], aT_ps.ap[1], [0, 32]],
            )
            nc.vector.tensor_copy(out=alpha_rep[:, sc, :, :], in_=aT_bcast)

        # ---- weighted sum, 4 slots packed in PE column groups, ragged K ----
        for g in range(NG):
            for nh in range(NH):
                ws_ps = ws_psum.tile([P, 512], F32, tag="ws")
                for bb in range(4):
                    j = 4 * g + bb
                    nchunk = FC[j] + (1 if REM[j] else 0)
                    for c in range(nchunk):
                        k = min(P, VB[j] - P * c)
                        ci = FO[j] + c if c < FC[j] else FCT + j
                        nc.tensor.matmul(
                            ws_ps[32 * bb : 32 * bb + 32, :],
                            lhsT=alpha_rep[0:k, c, j, :],
                            rhs=h_nat[0:k, ci, nh * 512 : (nh + 1) * 512],
                            start=(c == 0),
                            stop=(c == nchunk - 1),
                            tile_position=(0, 32 * bb),
                        )
                o_sc = rows.tile([P, 512], F32, tag="orow")
                nc.vector.tensor_copy(out=o_sc, in_=ws_ps)
                nc.gpsimd.dma_start(
                    out=out_ap[4 * g : 4 * g + 4, nh * 512 : (nh + 1) * 512],
                    in_=o_sc.rearrange("(b r) s -> b r s", r=32)[:, 0, :],
                )

    for _rep in range(reps):
        emit()


_CACHE = {}


def build(reps=1, plan=None):
    key = ("nc", reps, plan["WG"], plan["VB"])
    if key in _CACHE:
        return _CACHE[key]
    nc = bacc.Bacc("TRN2", target_bir_lowering=False, debug=False)
    with tile.TileContext(nc) as tc:
        with ExitStack() as ctx:
            tc._ctx = ctx
            _body(tc, reps, plan["WG"], plan["VB"])
    nc.compile()
    _CACHE[key] = nc
    return nc


def _prep_core_inputs(h, h_mask, ht, w1_w, w1_b, u_w, plan=None):
    """Host-side sharding + layout prep. Returns list of 8 per-core dicts."""
    if plan is None:
        plan = plan_for_mask(h_mask)
    order, counts, VB, WG = plan["order"], plan["counts"], plan["VB"], plan["WG"]
    SQmax = max(WG)
    FC = [v // P for v in VB]
    REM = [VB[j] - P * FC[j] for j in range(BL)]
    FO = np.concatenate([[0], np.cumsum(FC)]).astype(int)
    FCT = int(FO[-1])
    PO = np.concatenate([[0], np.cumsum(REM)]).astype(int)
    bf = ml_dtypes.bfloat16
    h_f = np.asarray(h, dtype=np.float32)
    ht_f = np.asarray(ht, dtype=np.float32)
    mask = np.asarray(h_mask) != 0

    w1t = np.asarray(w1_w[:, :H2], np.float32).T.astype(NP_F8)  # [H2, ATT]
    w2t = np.asarray(w1_w[:, H2:], np.float32).T.astype(NP_F8)
    u_col = np.ascontiguousarray(
        np.repeat(
            np.asarray(u_w[0], dtype=np.float32).reshape(TT, P).T[:, :, None],
            32,
            axis=2,
        )
    ).astype(bf)
    w1b_col = np.ascontiguousarray(
        np.asarray(w1_b, dtype=np.float32).reshape(TT, P).T
    ).astype(np.float32)

    in_maps = []
    for core in range(NCORES):
        # fused weights [P, KC, WF]: per k-chunk c -> w1 | w2 | htT columns
        wts = np.zeros((P, KC, WF), dtype=NP_F8)
        h_t = np.zeros((P, KC, 4 * (WG[0] + WG[1])), dtype=NP_F8)
        h_natf = np.zeros((P, FCT, H2), dtype=bf)
        h_natp = np.zeros((int(PO[-1]), H2), dtype=bf)
        maskadd = np.full((BL, SQmax), -1.0e20, dtype=np.float32)
        ht_sel = np.empty((BL, T, H2), dtype=np.float32)
        w1_3 = w1t.reshape(KC, P, ATT).transpose(1, 0, 2)  # [P, KC, ATT]
        w2_3 = w2t.reshape(KC, P, ATT).transpose(1, 0, 2)
        wts[:, :, 0:ATT] = w1_3
        wts[:, :, ATT : 2 * ATT] = w2_3
        for j in range(BL):
            b = order[8 * j + core]
            idx = np.flatnonzero(mask[b])
            v = len(idx)
            assert v <= VB[j], f"core {core} slot {j}: {v} > VB {VB[j]}"
            hc = h_f[b, idx]  # [v, H2]
            g, bb = j // 4, j % 4
            off = 4 * KC * WG[0] * g  # within h_t free dim, as [KC, 4, WG]
            blk = h_t[:, :, :].reshape(P, -1)
            W = WG[g]
            # hcT8 [H2, v] -> [P, KC, v] partition-major
            hcT8 = hc.astype(NP_F8).T.reshape(KC, P, v).transpose(1, 0, 2)
            base = off
            view = blk[:, base : base + KC * 4 * W].reshape(P, KC, 4, W)
            view[:, :, bb, :v] = hcT8
            hb = np.zeros((VB[j], H2), dtype=bf)
            hb[:v] = hc.astype(bf)
            for c in range(FC[j]):
                h_natf[:, FO[j] + c, :] = hb[c * P : (c + 1) * P]
            if REM[j]:
                h_natp[PO[j] : PO[j] + REM[j]] = hb[FC[j] * P :]
            maskadd[j, :v] = 0.0
            ht_sel[j] = ht_f[b]
        htT = ht_sel.reshape(BL * T, H2).T.astype(NP_F8)  # [H2, BL*T]
        wts[:, :, 2 * ATT : WF] = htT.reshape(KC, P, BL * T).transpose(1, 0, 2)
        in_maps.append(
            {
                "h_t": np.ascontiguousarray(h_t.reshape(P, -1)),
                "h_natf": np.ascontiguousarray(h_natf.reshape(P, -1)),
                "h_natp": np.ascontiguousarray(h_natp),
                "wts": np.ascontiguousarray(wts.reshape(P, -1)),
                "u_col": u_col,
                "w1b_col": w1b_col,
                "maskadd": maskadd,
            }
        )
    return in_maps


def kernel(h, h_mask, ht, w1_w, w1_b, u_w):
    plan = plan_for_mask(h_mask)
    nc = build(plan=plan)
    in_maps = _prep_core_inputs(h, h_mask, ht, w1_w, w1_b, u_w, plan=plan)
    res = bass_utils.run_bass_kernel_spmd(
        nc,
        in_maps,
        core_ids=list(range(NCORES)),
        trace=bool(int(os.environ.get("KERNEL_TRACE", "0"))),
    )
    _CACHE["last_result"] = res
    out = np.empty((B, H2), dtype=np.float32)
    order = plan["order"]
    for core in range(NCORES):
        for j in range(BL):
            out[order[8 * j + core]] = res.results[core]["out"][j]
    return np.ascontiguousarray(out)
